# revision 11
# baseline (speedup 1.0000x reference)
"""Trainium2 Bass kernel for StyleGAN2-style upsampled Conv1d ("q-form").

Reference computation (x:(16,256,4096), w:(256,256,3), bias:(256,)):
  y = conv_transpose1d(x, w, stride=2); z = upfirdn1d(y, [1,3,3,1]/8*2)
  out = z + bias                                   # (16, 256, 8192)

Direct form folds the FIR into 6 effective tap matrices (K=1536 per
output pair -> 196k PE rows/core, ~83us).  This kernel instead computes
the three RAW tap products once per input position
    q_t[i] = w_t^T xp[i]          (xp = x zero-padded by 1 each side)
which is only K=768 per output pair -> 98k PE rows/core (~41us), and
reconstructs the two output phases with cheap shifted adds (u = window-
local output index, v = u relative q index):
    even = .75*E1 + .25*E2 + b    E1 = q0[u]  +q1[u+1]+q2[u+1]
    odd  = .75*O1 + .25*O2 + b    E2 = q1[u]  +q0[u+1]+q2[u+2]
                                  O1 = q0[u+1]+q1[u+1]+q2[u+2]
                                  O2 = q0[u]  +q1[u+2]+q2[u+1]
with shared subexpressions  s2 = q0[u]+q2[u+1],  s1 = q0[u+1]+q2[u+2]:
    E1 = s2+q1[u+1]   O2 = s2+q1[u+2]   E2 = s1+q1[u]   O1 = s1+q1[u+1]
The 10 elementwise ops per 510-position window are spread over the
vector (DVE), gpsimd (Pool) and scalar (ACT) engines, all of which can
read PSUM directly.  pre = E2/3 + E1 via scalar_tensor_tensor, then
out = .75*pre + bias via tensor_scalar / activation.

Even and odd outputs are written as separate contiguous phase PLANES
(out[b, phase, ch, j]); the host interleaves them into the final
(b, ch, 8192) layout.  This avoids the 2-strided on-chip writes that
run at ~1.3 ns/col instead of ~0.5.  Stores are batched over pairs of
windows (1020 cols) to halve DMA-issue instruction overhead.

Everything on-chip is fp16 (x, w, q drains, intermediates, output
planes; PSUM and bias fp32): fp16 matmuls run at the same 1 row/cycle
as bf16 but carry 10 mantissa bits, and 16-bit elementwise ops run in
the DVE's 2x mode.

Sharding: data-parallel over batch (2 per core x 8 cores).
"""

import numpy as np

import concourse.bass as bass
import concourse.mybir as mybir
import concourse.tile as tile
from concourse import bacc
from concourse.bass_utils import run_bass_kernel_spmd

N, IN_CH, OUT_CH, KERNEL, D = 16, 256, 256, 3, 4096
NCORES = 8
BPC = N // NCORES          # batches per core
DOUT = 2 * D
F32 = mybir.dt.float32
F16 = mybir.dt.float16

WIN = 510                  # output positions per full window
NFULL = D // WIN           # 8 full windows ...
TAILW = D - NFULL * WIN    # ... + one 16-output tail window per (bb, m)
NWARM = 10                 # leading dummy matmuls: clock ramp while DMAs land
NTAIL = 6                  # trailing dummies: hold the clock through epilogue

AF_ID = mybir.ActivationFunctionType.Identity
ADD = mybir.AluOpType.add
MULT = mybir.AluOpType.mult

_CACHED = {}


def _wblk(tap, k, m):
    # m-major so the m=0 half of the weights is one early contiguous DMA
    return (m * 3 + tap) * 2 + k


def _build_nc():
    nc = bacc.Bacc("TRN2", target_bir_lowering=False, debug=False)

    x_t = nc.dram_tensor("x", [BPC, IN_CH, D + 2], F16, kind="ExternalInput")
    # w layout: 12 blocks of (128 K, 128 M); see _wblk
    w_t = nc.dram_tensor("w", [128, 12 * 128], F16, kind="ExternalInput")
    b_t = nc.dram_tensor("b", [128, 2], F32, kind="ExternalInput")
    # phase-plane output; host interleaves even/odd
    o_t = nc.dram_tensor("out", [BPC, 2, OUT_CH, D], F16, kind="ExternalOutput")

    with tile.TileContext(nc) as tc:
        with (
            tc.tile_pool(name="wpool", bufs=1) as wpool,
            tc.tile_pool(name="xpool", bufs=2 * BPC) as xpool,
            tc.tile_pool(name="epool", bufs=4) as epool,
            tc.tile_pool(name="zpool", bufs=3) as zpool,
            tc.tile_pool(name="qpool", bufs=2, space="PSUM") as qpool,
            tc.tile_pool(name="wmpool", bufs=1, space="PSUM") as wmpool,
        ):
            w_sb = wpool.tile([128, 12 * 128], F16)
            b_sb = wpool.tile([128, 2], F32)
            x_sb = {}
            for bb in range(BPC):
                for k in range(2):
                    x_sb[bb, k] = xpool.tile(
                        [128, D + 2], F16, tag="x", name=f"x_{bb}_{k}"
                    )

            # Warmup operand memset first: gpsimd exits the framework
            # prologue earliest, and 200ns here buys an earlier PE ramp.
            warm_f = wpool.tile([128, 128 + 512], F16)
            nc.gpsimd.memset(warm_f[:], 1.0)

            # Single SWDGE stream in exact first-use order (early DMA bw is
            # one shared ~400 GB/s pool; splitting queues only lets
            # non-critical bytes steal from critical ones).
            nc.scalar.dma_start(out=b_sb[:], in_=b_t[:])
            nc.gpsimd.dma_start(out=w_sb[:, 0:1536], in_=w_t[:, 0:1536])
            head = 2 * WIN + 2
            for k in range(2):
                nc.gpsimd.dma_start(
                    out=x_sb[0, k][:, 0:head],
                    in_=x_t[0, k * 128:(k + 1) * 128, 0:head],
                )
            cuts = list(range(head, D + 2, 1022)) + [D + 2]
            for t in range(len(cuts) - 1):
                for k in range(2):
                    lo, hi = cuts[t], cuts[t + 1]
                    nc.gpsimd.dma_start(
                        out=x_sb[0, k][:, lo:hi],
                        in_=x_t[0, k * 128:(k + 1) * 128, lo:hi],
                    )
            for k in range(2):
                nc.gpsimd.dma_start(
                    out=x_sb[1, k][:], in_=x_t[1, k * 128:(k + 1) * 128, :]
                )

            # PE warmup while the first DMAs land: dummy fp16 matmuls run
            # the HAM clock-gate ramp so the real stream starts near the
            # full 2.4 GHz; results land in a scratch PSUM bank.
            warm_ps = wmpool.tile([128, 512], F32, tag="wm", name="warm_ps")
            for _ in range(NWARM):
                nc.tensor.matmul(
                    warm_ps[:],
                    lhsT=warm_f[:, 0:128],
                    rhs=warm_f[:, 128:640],
                    start=True,
                    stop=True,
                )

            def window(bb, m, c, wcols, nout, ze, zo, zoff):
                # q tile: [q0 | q1 | q2], each in its own PSUM bank
                q = qpool.tile([128, 3 * 512], F32, tag="q",
                               name=f"q_{bb}_{m}_{c}")
                x0 = WIN * c
                for t in range(3):
                    for k in range(2):
                        nc.tensor.matmul(
                            q[:, t * 512:t * 512 + wcols],
                            lhsT=w_sb[:, _wblk(t, k, m) * 128:][:, :128],
                            rhs=x_sb[bb, k][:, x0:x0 + wcols],
                            start=(k == 0),
                            stop=(k == 1),
                        )
                q0 = q[:, 0:wcols]
                q1 = q[:, 512:512 + wcols]
                q2 = q[:, 1024:1024 + wcols]

                sfx = f"_{bb}_{m}_{c}"
                n = nout
                s1 = epool.tile([128, WIN], F16, tag="s1", name="s1" + sfx)
                s2 = epool.tile([128, WIN], F16, tag="s2", name="s2" + sfx)
                q2d = epool.tile([128, WIN + 2], F16, tag="q2d", name="q2d" + sfx)
                e1 = epool.tile([128, WIN], F16, tag="e1", name="e1" + sfx)
                e2 = epool.tile([128, WIN], F16, tag="e2", name="e2" + sfx)
                o1 = epool.tile([128, WIN], F16, tag="o1", name="o1" + sfx)
                o2 = epool.tile([128, WIN], F16, tag="o2", name="o2" + sfx)
                pe = epool.tile([128, WIN], F16, tag="pe", name="pe" + sfx)
                po = epool.tile([128, WIN], F16, tag="po", name="po" + sfx)
                bias_ap = b_sb[:, m:m + 1]

                # PSUM rules: one PSUM operand per instruction, and gpsimd
                # (Pool) cannot touch PSUM at all.  So: drain q to fp16
                # SBUF (vector copies + scalar act), then run every add in
                # fp16 2x mode with gpsimd carrying the odd-phase half.
                q0d = epool.tile([128, WIN + 2], F16, tag="q0d", name="q0d" + sfx)
                q1d = epool.tile([128, WIN + 2], F16, tag="q1d", name="q1d" + sfx)
                nc.vector.tensor_copy(out=q0d[:, :n + 2], in_=q0[:, 0:n + 2])
                nc.vector.tensor_copy(out=q2d[:, :n + 2], in_=q2[:, 0:n + 2])
                nc.scalar.activation(
                    out=q1d[:, :n + 2], in_=q1[:, 0:n + 2], func=AF_ID)
                nc.gpsimd.tensor_tensor(
                    out=s2[:, :n], in0=q0d[:, 0:n], in1=q2d[:, 1:n + 1], op=ADD)
                nc.gpsimd.tensor_tensor(
                    out=s1[:, :n], in0=q0d[:, 1:n + 1], in1=q2d[:, 2:n + 2], op=ADD)
                nc.vector.tensor_tensor(
                    out=e1[:, :n], in0=q1d[:, 1:n + 1], in1=s2[:, :n], op=ADD)
                nc.vector.tensor_tensor(
                    out=e2[:, :n], in0=q1d[:, 0:n], in1=s1[:, :n], op=ADD)
                nc.gpsimd.tensor_tensor(
                    out=o1[:, :n], in0=q1d[:, 1:n + 1], in1=s1[:, :n], op=ADD)
                nc.gpsimd.tensor_tensor(
                    out=o2[:, :n], in0=q1d[:, 2:n + 2], in1=s2[:, :n], op=ADD)
                # pre = E2/3 + E1 ; plane = .75*pre + bias
                nc.vector.scalar_tensor_tensor(
                    out=pe[:, :n], in0=e2[:, :n], scalar=1.0 / 3.0,
                    in1=e1[:, :n], op0=MULT, op1=ADD)
                nc.vector.scalar_tensor_tensor(
                    out=po[:, :n], in0=o2[:, :n], scalar=1.0 / 3.0,
                    in1=o1[:, :n], op0=MULT, op1=ADD)
                nc.vector.tensor_scalar(
                    out=ze[:, zoff:zoff + n], in0=pe[:, :n],
                    scalar1=0.75, scalar2=bias_ap, op0=MULT, op1=ADD)
                nc.scalar.activation(
                    out=zo[:, zoff:zoff + n], in_=po[:, :n], func=AF_ID,
                    bias=bias_ap, scale=0.75)

            for bb in range(BPC):
                zcur = {}
                for c in range(NFULL + 1):
                    wcols = 512 if c < NFULL else TAILW + 2
                    nout = WIN if c < NFULL else TAILW
                    for m in range(2):
                        if c % 2 == 0:
                            zcur[m] = (
                                zpool.tile([128, 2 * WIN], F16, tag=f"ze{m}",
                                           name=f"ze_{bb}_{m}_{c}"),
                                zpool.tile([128, 2 * WIN], F16, tag=f"zo{m}",
                                           name=f"zo_{bb}_{m}_{c}"),
                                c,
                            )
                        ze, zo, c0 = zcur[m]
                        window(bb, m, c, wcols, nout, ze, zo, (c - c0) * WIN)
                        if c % 2 == 1 or c == NFULL:
                            span = (c - c0) * WIN + nout
                            j0 = c0 * WIN
                            nc.sync.dma_start(
                                out=o_t[bb, 0, m * 128:(m + 1) * 128,
                                        j0:j0 + span],
                                in_=ze[:, :span])
                            nc.scalar.dma_start(
                                out=o_t[bb, 1, m * 128:(m + 1) * 128,
                                        j0:j0 + span],
                                in_=zo[:, :span])

            # Trailing dummies hold the clock while final drains retire.
            for _ in range(NTAIL):
                nc.tensor.matmul(
                    warm_ps[:],
                    lhsT=warm_f[:, 0:128],
                    rhs=warm_f[:, 128:640],
                    start=True,
                    stop=True,
                )
    nc.compile()
    return nc


def _host_weights(weight, bias):
    w = np.asarray(weight, dtype=np.float32)
    w_host = np.zeros((128, 12 * 128), dtype=np.float32)
    for t in range(3):
        for k in range(2):
            for m in range(2):
                blk = _wblk(t, k, m)
                wt = w[m * 128:(m + 1) * 128, k * 128:(k + 1) * 128, t]
                w_host[:, blk * 128:(blk + 1) * 128] = wt.T
    b_host = np.asarray(bias, dtype=np.float32).reshape(2, 128).T.copy()
    return w_host.astype(np.float16), b_host


def _host_x(x):
    x = np.asarray(x, dtype=np.float32)
    xp = np.pad(x, ((0, 0), (0, 0), (1, 1)))
    return np.ascontiguousarray(xp.astype(np.float16))


def _host_out(results):
    # results: list of per-core dicts with "out" [BPC, 2, OUT_CH, D] fp16
    planes = np.concatenate(
        [np.asarray(r["out"]) for r in results], axis=0
    ).astype(np.float32)
    out = np.empty((N, OUT_CH, DOUT), dtype=np.float32)
    out[:, :, 0::2] = planes[:, 0]
    out[:, :, 1::2] = planes[:, 1]
    return out


def kernel(x, weight, bias):
    x = _host_x(x)
    w_host, b_host = _host_weights(weight, bias)

    if "nc" not in _CACHED:
        _CACHED["nc"] = _build_nc()
    nc = _CACHED["nc"]

    in_maps = []
    for core in range(NCORES):
        shard = np.ascontiguousarray(x[core * BPC:(core + 1) * BPC])
        in_maps.append({"x": shard, "w": w_host, "b": b_host})

    res = run_bass_kernel_spmd(nc, in_maps, core_ids=list(range(NCORES)))
    return _host_out(res.results)


# revision 15
# speedup vs baseline: 2.3055x; 2.3055x over previous
"""Trainium2 Bass kernel for StyleGAN2-style upsampled Conv1d.

Reference computation (for x:(16,256,4096), weight:(256,256,3), bias:(256,)):
  y = conv_transpose1d(x, weight, stride=2)      # correlation on 2x-dilated x
  z = upfirdn1d(y, [1,3,3,1]/8 * 2)              # depthwise FIR
  out = z + bias                                  # (16, 256, 8192)

The transposed conv + FIR collapse into TWO 3-tap correlations over the
original x grid (even/odd output phases):
  out[:, :, 2j]   = A @x[j-1] + B @x[j]  + C @x[j+1]
  out[:, :, 2j+1] = A'@x[j-1] + B'@x[j]  + C'@x[j+1]
with (w0,w1,w2) = weight taps:
  A  = .75 w0 + .25 w1   B  = .25 w0 + .75 w1 + .75 w2   C  = .25 w2
  A' = .25 w0            B' = .75 w0 + .75 w1 + .25 w2   C' = .25 w1 + .75 w2

On-chip: chunk-major streaming — for each NCHUNK-position chunk, 12
accumulating fp32r matmuls (2 phases x 3 taps x 2 K-tiles) fill one
PSUM pair, which drains immediately (bias add + even/odd interleave in
one vector or scalar op) and DMAs out.  Draining each pair right after
its matmuls keeps the PSUM pool from ever stalling the PE.  Sharding:
data-parallel over batch (2 per core x 8 cores).

DMA model (measured): every queue has ~2.5us issue-to-data latency;
SWDGE sustains ~390 GB/s, each HWDGE queue ~190 GB/s, all sharing
~400-450 GB/s of HBM.  So the critical first tiles (x chunk-0 heads +
m=0 weights, ~1.3 MB) are split across all three queues in parallel
and everything else queues strictly behind them.
"""

import numpy as np

import concourse.bass as bass
import concourse.mybir as mybir
import concourse.tile as tile
from concourse import bacc
from concourse.bass_utils import run_bass_kernel_spmd

N, IN_CH, OUT_CH, KERNEL, D = 16, 256, 256, 3, 4096
NCORES = 8
BPC = N // NCORES          # batches per core
DOUT = 2 * D
F32 = mybir.dt.float32
F32R = mybir.dt.float32r

BF16 = mybir.dt.bfloat16

NCHUNK = 512               # matmul moving free dim (= one PSUM bank of fp32)
NCHUNKS = D // NCHUNK
HEAD = NCHUNK + 3          # x columns needed by chunk 0
NWARM = 10                 # leading dummy matmuls: clock ramp while DMAs land
NTAIL = 8                  # trailing dummies: hold the clock through epilogue

_CACHED = {}


def _wblk(phase, tap, k, m):
    # m-major so each m-half of the weights is one contiguous DMA
    return ((m * 2 + phase) * 3 + tap) * 2 + k


def _build_nc(mm_dtype=BF16):
    nc = bacc.Bacc("TRN2", target_bir_lowering=False, debug=False)

    # x arrives host-padded with zero columns at 0 and D+1 (3-tap halo),
    # pre-cast to bf16 (tolerance is 2e-2; bf16 rounding costs ~4e-3).
    # bf16 halves every DMA byte, halves LDWEIGHTS/SBUF traffic, and the
    # PE still accumulates in fp32 PSUM at the same 1 row/cycle.
    x_t = nc.dram_tensor("x", [BPC, IN_CH, D + 2], mm_dtype, kind="ExternalInput")
    # w layout: 24 blocks of (128 K, 128 M); see _wblk
    w_t = nc.dram_tensor("w", [128, 24 * 128], mm_dtype, kind="ExternalInput")
    b_t = nc.dram_tensor("b", [128, 2], F32, kind="ExternalInput")
    # Output leaves the chip as bf16 (half the store bytes); the host
    # upcasts back to fp32.
    o_t = nc.dram_tensor("out", [BPC, OUT_CH, DOUT], BF16, kind="ExternalOutput")

    pbufs = 8 * 512 // (2 * NCHUNK)   # PSUM pairs that fit in 8 banks
    with tile.TileContext(nc) as tc:
        with (
            tc.tile_pool(name="wpool", bufs=1) as wpool,
            tc.tile_pool(name="xpool", bufs=2 * BPC) as xpool,
            tc.tile_pool(name="zpool", bufs=2 * pbufs) as zpool,
            tc.tile_pool(name="ppool", bufs=pbufs, space="PSUM") as ppool,
        ):
            w_sb = wpool.tile([128, 24 * 128], mm_dtype)
            b_sb = wpool.tile([128, 2], F32)
            x_sb = {}
            for bb in range(BPC):
                for k in range(2):
                    x_sb[bb, k] = xpool.tile(
                        [128, D + 2], mm_dtype, tag="x", name=f"x_{bb}_{k}"
                    )

            # Measured: early DMA bandwidth is ONE shared ~400 GB/s pool
            # with a ~2us issue-to-data lag — splitting across queues
            # does not add bandwidth, it only lets non-critical bytes
            # steal from critical ones.  So every input rides the single
            # SWDGE stream in exact first-use order; HWDGE queues carry
            # only the 1KB bias and, later, the output stores.
            # Warmup operand memset first: gpsimd exits the framework
            # prologue earliest; 200ns here buys ~1.4us of earlier HAM ramp.
            warm_bf = wpool.tile([128, 128 + 512], mybir.dt.bfloat16)
            nc.gpsimd.memset(warm_bf[:], 1.0)

            nc.scalar.dma_start(out=b_sb[:], in_=b_t[:])
            for k in range(2):
                nc.gpsimd.dma_start(
                    out=x_sb[0, k][:, 0:HEAD],
                    in_=x_t[0, k * 128:(k + 1) * 128, 0:HEAD],
                )
            nc.gpsimd.dma_start(out=w_sb[:, 0:768], in_=w_t[:, 0:768])
            nc.gpsimd.dma_start(out=w_sb[:, 768:1536], in_=w_t[:, 768:1536])
            cuts = list(range(HEAD, D + 2, 515)) + [D + 2]
            for t in range(len(cuts) - 1):
                for k in range(2):
                    lo, hi = cuts[t], cuts[t + 1]
                    nc.gpsimd.dma_start(
                        out=x_sb[0, k][:, lo:hi],
                        in_=x_t[0, k * 128:(k + 1) * 128, lo:hi],
                    )
            nc.gpsimd.dma_start(out=w_sb[:, 1536:3072], in_=w_t[:, 1536:3072])
            for k in range(2):
                nc.gpsimd.dma_start(
                    out=x_sb[1, k][:], in_=x_t[1, k * 128:(k + 1) * 128, :]
                )

            # PE warmup while the first DMAs land: dummy bf16 matmuls on
            # a memset tile run the HAM clock-gate ramp (~3us + margin)
            # so the real stream starts at the full 2.4 GHz.  The PSUM
            # garbage lands in a pool slot that a later real
            # accumulation group's start=True clears.
            warm_ps = ppool.tile([128, 2 * NCHUNK], F32, tag="pair", name="warm_ps")
            for _ in range(NWARM):
                nc.tensor.matmul(
                    warm_ps[:, 0:512],
                    lhsT=warm_bf[:, 0:128],
                    rhs=warm_bf[:, 128:640],
                    start=True,
                    stop=True,
                )

            # Chunk list: (bb, m, x0, width).  The very last 512-chunk is
            # split into two 256-halves so the final drain+store after the
            # last matmul covers 4x less data (256-row matmuls still beat
            # the ~97ns LDWEIGHTS floor, so the PE loses nothing).
            seq = []
            for bb in range(BPC):
                for m in range(2):
                    lastrow = bb == BPC - 1 and m == 1
                    for c in range(NCHUNKS):
                        if lastrow and c == NCHUNKS - 1:
                            seq.append((bb, m, c * NCHUNK, NCHUNK // 2))
                            seq.append((bb, m, c * NCHUNK + NCHUNK // 2,
                                        NCHUNK // 2))
                        else:
                            seq.append((bb, m, c * NCHUNK, NCHUNK))

            # Store issues round-robin over three queues (sync/scalar
            # HWDGE + gpsimd SWDGE) so no engine's issue chain delays the
            # final drains.
            for ci, (bb, m, x0, width) in enumerate(seq):
                pair = ppool.tile([128, 2 * width], F32, tag="pair",
                                  name=f"pair_{bb}_{m}_{x0}")
                for phase in range(2):
                    for tap in range(3):
                        for k in range(2):
                            w_ap = w_sb[:, _wblk(phase, tap, k, m) * 128:][:, :128]
                            rhs = x_sb[bb, k][:, x0 + tap:x0 + tap + width]
                            nc.tensor.matmul(
                                pair[:, phase * width:(phase + 1) * width],
                                lhsT=w_ap,
                                rhs=rhs,
                                start=(tap == 0 and k == 0),
                                stop=(tap == 2 and k == 1),
                            )
                bias_ap = b_sb[:, m:m + 1]
                zt = zpool.tile([128, 2 * width], BF16, tag="z",
                                name=f"z_{bb}_{m}_{x0}")
                # psum pair is [even(width) | odd(width)]; writing in
                # (phase, j) order at stride 2 interleaves the two
                # phases while adding bias.
                last = ci == len(seq) - 1
                vin = pair[:].rearrange("p (two j) -> p two j", two=2)
                if not last:
                    vout = zt[:].rearrange("p (j two) -> p two j", two=2)
                    if ci % 2 == 0:
                        nc.vector.tensor_scalar(
                            out=vout, in0=vin,
                            scalar1=bias_ap, scalar2=None,
                            op0=mybir.AluOpType.add,
                        )
                    else:
                        nc.scalar.activation(
                            out=vout, in_=vin,
                            func=mybir.ActivationFunctionType.Identity,
                            bias=bias_ap,
                        )
                    oeng = (nc.sync, nc.scalar, nc.gpsimd)[ci % 3]
                    oeng.dma_start(
                        out=o_t[bb, m * 128:(m + 1) * 128,
                                2 * x0:2 * x0 + 2 * width],
                        in_=zt[:],
                    )
                else:
                    # Final (quarter-size) chunk: drain halves concurrently
                    # on vector+scalar, store via both HWDGE queues.
                    for h, dma_eng in enumerate((nc.sync, nc.scalar)):
                        vout = zt[:, h * width:(h + 1) * width].rearrange(
                            "p (j two) -> p two j", two=2
                        )
                        vin_h = vin[:, :, h * (width // 2):(h + 1) * (width // 2)]
                        if h == 0:
                            nc.vector.tensor_scalar(
                                out=vout, in0=vin_h,
                                scalar1=bias_ap, scalar2=None,
                                op0=mybir.AluOpType.add,
                            )
                        else:
                            nc.scalar.activation(
                                out=vout, in_=vin_h,
                                func=mybir.ActivationFunctionType.Identity,
                                bias=bias_ap,
                            )
                        dma_eng.dma_start(
                            out=o_t[bb, m * 128:(m + 1) * 128,
                                    2 * x0 + h * width:
                                    2 * x0 + (h + 1) * width],
                            in_=zt[:, h * width:(h + 1) * width],
                        )

            # Trailing dummies keep the PE active (and the clock gate at
            # 8/8) while the final drains + stores retire, so the
            # framework's epilogue barrier doesn't run at 1/2 clock.
            tail_ps = ppool.tile([128, 2 * NCHUNK], F32, tag="pair",
                                 name="tail_ps")
            for _ in range(NTAIL):
                nc.tensor.matmul(
                    tail_ps[:, 0:512],
                    lhsT=warm_bf[:, 0:128],
                    rhs=warm_bf[:, 128:640],
                    start=True,
                    stop=True,
                )
    nc.compile()
    return nc


def _host_weights(weight, bias):
    w = np.asarray(weight, dtype=np.float32)
    w0, w1, w2 = w[:, :, 0], w[:, :, 1], w[:, :, 2]
    taps = [
        [0.75 * w0 + 0.25 * w1, 0.25 * w0 + 0.75 * w1 + 0.75 * w2, 0.25 * w2],
        [0.25 * w0, 0.75 * w0 + 0.75 * w1 + 0.25 * w2, 0.25 * w1 + 0.75 * w2],
    ]
    w_host = np.zeros((128, 24 * 128), dtype=np.float32)
    for phase in range(2):
        for tap in range(3):
            for k in range(2):
                for m in range(2):
                    blk = _wblk(phase, tap, k, m)
                    # lhsT block[i, o] = W[phase][tap][m*128+o, k*128+i]
                    wt = taps[phase][tap][m * 128:(m + 1) * 128, k * 128:(k + 1) * 128]
                    w_host[:, blk * 128:(blk + 1) * 128] = wt.T
    b_host = np.asarray(bias, dtype=np.float32).reshape(2, 128).T.copy()
    import ml_dtypes
    return w_host.astype(ml_dtypes.bfloat16), b_host


def _host_x(x):
    import ml_dtypes
    x = np.asarray(x, dtype=np.float32)
    xp = np.pad(x, ((0, 0), (0, 0), (1, 1)))
    return np.ascontiguousarray(xp.astype(ml_dtypes.bfloat16))


def _host_out(results):
    return np.concatenate(
        [np.asarray(r["out"]).astype(np.float32) for r in results], axis=0
    )


def kernel(x, weight, bias):
    x = _host_x(x)
    w_host, b_host = _host_weights(weight, bias)

    if "nc" not in _CACHED:
        _CACHED["nc"] = _build_nc()
    nc = _CACHED["nc"]

    in_maps = []
    for core in range(NCORES):
        shard = np.ascontiguousarray(x[core * BPC:(core + 1) * BPC])
        in_maps.append({"x": shard, "w": w_host, "b": b_host})

    res = run_bass_kernel_spmd(nc, in_maps, core_ids=list(range(NCORES)))
    return _host_out(res.results)

